# revision 38
# baseline (speedup 1.0000x reference)
"""Causal self-attention (B=4, T=2048, E=768, H=12, D=64) on 8 TRN2 NeuronCores.

Sharding: core c handles batch b = c//2 and head-group g = c%2 (6 heads each).
Per core:
    qT, kT = (x @ WqT + bq).T, ...        stored [384, 2048] (heads x 64, T)
    v      = x @ WvT + bv                 stored [2048, 6, 65] with a ones col
    per head-pair, per key-block sb (128 keys), per 512-col t piece:
        scoresT[s, t] = kT_h[:, s]·qT_h[:, t] for BOTH heads into ONE psum
        tile [128, 1024] (h0 at cols 0-511, h1 at 512-1023). The two matmuls
        are adjacent and touch disjoint PE row groups (rows 0-63 / 64-127),
        so the systolic array runs them concurrently (measured 2x).
        ONE exp op covers both heads' halves (same per-key mask bias), writing
        a pair-interleaved bf16 expT buffer.
    per head, per 512-col t-chunk:
        outT[d_aug, t] += v_aug[s, :].T @ expT[s, t]  (PE accumulate, M=65)
    outT row 64 = softmax denominator (ones column of v_aug).
Host: output[b, :, h*64:(h+1)*64] = (outT_h[:64] / outT_h[64:65]).T

Tail pieces of each key strip are extended backward to a full 512 columns
(recomputing a small overlap) so every psum piece is full -> exp ops stay
1024 wide. Strips shorter than 512 use two exp ops (gap in psum).
All matmul writes start on a PSUM bank boundary (mid-bank start=True writes
hang the hardware). Inputs are host-packed into the exact SBUF layouts so
every input lands in a handful of contiguous 2D DMAs on two queues.
"""

import numpy as np
import ml_dtypes

import concourse.bacc as bacc
import concourse.mybir as mybir
import concourse.tile as tile
from concourse import bass_utils

F32 = mybir.dt.float32
BF16 = mybir.dt.bfloat16

B, T, E, H, D = 4, 2048, 768, 12, 64
NCORES = 8
HPC = 6             # heads per core
OC = HPC * D        # 384 output channels per core
ECH = E // 128      # 6 contraction chunks
QKC = OC // 128     # 3 qT/kT partition chunks (= head pairs)
NSB = T // 128      # 16 key blocks
SCALE = 0.125       # 1/sqrt(D)
TCH = 512           # PV t-chunk width
XSL = ECH * 512     # xT free elems per t-slice (3072)


def _strip_pieces(sb):
    """Non-overlapping (start, width) pieces of strip sb on the 512 grid."""
    W = T - 128 * sb
    return [(p, min(512, W - p)) for p in range(0, W, 512)]


# pair-interleaved expT buffer layout: strip sb at PAIR_OFF[sb]; piece p at
# PAIR_OFF[sb] + 1024*p, holding h0's pw cols then h1's pw cols
PAIR_OFF = [0] * (NSB + 1)
for _sb in range(NSB):
    PAIR_OFF[_sb + 1] = PAIR_OFF[_sb] + 2 * (T - 128 * _sb)
PAIRW = PAIR_OFF[NSB]  # 34816


def _xoff(e, t):
    """Free-dim offset of (e-chunk, t) in the slice-major packed xT tile.
    Valid for ranges within one 512-wide t slice."""
    return (t // 512) * XSL + 512 * e + (t % 512)


def _build():
    nc = bacc.Bacc("TRN2", debug=False)

    xT_d = nc.dram_tensor("xTp", [128, ECH * T], BF16, kind="ExternalInput")
    wq_d = nc.dram_tensor("wqp", [128, ECH * OC], BF16, kind="ExternalInput")
    wk_d = nc.dram_tensor("wkp", [128, ECH * OC], BF16, kind="ExternalInput")
    wv_d = nc.dram_tensor("wvp", [128, ECH * OC], BF16, kind="ExternalInput")
    bq_d = nc.dram_tensor("bq", [QKC, 128, 1], F32, kind="ExternalInput")
    bk_d = nc.dram_tensor("bk", [QKC, 128, 1], F32, kind="ExternalInput")
    bvr_d = nc.dram_tensor("bvr", [128, OC], F32, kind="ExternalInput")
    mb_d = nc.dram_tensor("mb", [128, NSB], F32, kind="ExternalInput")
    tri_d = nc.dram_tensor("tri", [128, 128], BF16, kind="ExternalInput")
    out_d = nc.dram_tensor("outT", [HPC, D + 1, T], F32, kind="ExternalOutput")

    with tile.TileContext(nc) as tc:
        with (
            tc.tile_pool(name="persist", bufs=1) as pp,
            tc.tile_pool(name="qk_ps", bufs=3, space="PSUM") as qk_ps,
            tc.tile_pool(name="b1_ps", bufs=2, space="PSUM") as b1_ps,
            tc.tile_pool(name="stage", bufs=4) as stage,
        ):
            # ---- persistent SBUF tensors ----
            xt_all = pp.tile([128, ECH * T], BF16, tag="xt", name="xt")
            wq_all = pp.tile([128, ECH * OC], BF16, tag="wq", name="wq")
            wk_all = pp.tile([128, ECH * OC], BF16, tag="wk", name="wk")
            wv_all = pp.tile([128, ECH * OC], BF16, tag="wv", name="wv")
            # wq/wk are packed chunk-major: lhsT for (c, e) at cols 768c+128e
            wv = [wv_all[:, OC * e:OC * (e + 1)] for e in range(ECH)]

            def wqk_sl(w_all, c, e):
                o = 768 * c + 128 * e
                return w_all[:, o:o + 128]
            qt = [pp.tile([128, T], BF16, tag=f"qt{c}", name=f"qt{c}") for c in range(QKC)]
            kt = [pp.tile([128, T], BF16, tag=f"kt{c}", name=f"kt{c}") for c in range(QKC)]
            vt = [pp.tile([128, HPC, D + 1], BF16, tag=f"vt{t}", name=f"vt{t}") for t in range(NSB)]
            exb = pp.tile([128, PAIRW], BF16, tag="exb", name="exb")
            bq_t = [pp.tile([128, 1], F32, tag=f"bq{c}", name=f"bq{c}") for c in range(QKC)]
            bk_t = [pp.tile([128, 1], F32, tag=f"bk{c}", name=f"bk{c}") for c in range(QKC)]
            bvr_t = pp.tile([128, OC], F32, tag="bvr", name="bvr")
            mb_t = pp.tile([128, NSB], F32, tag="mb", name="mb")
            tri_t = pp.tile([128, 128], BF16, tag="tri", name="tri")

            # ---- input DMAs: contiguous 2D transfers on two hardware queues
            # (sync + scalar); xT slice-major so projections start early ----
            for c in range(QKC):
                nc.sync.dma_start(wk_all[:, 768 * c:768 * c + 768],
                                  wk_d.ap()[:, 768 * c:768 * c + 768])
                nc.sync.dma_start(wq_all[:, 768 * c:768 * c + 768],
                                  wq_d.ap()[:, 768 * c:768 * c + 768])
            for t0 in (0, 512, 1024, 1536):
                s0 = (t0 // 512) * XSL
                nc.scalar.dma_start(
                    xt_all[:, s0:s0 + XSL], xT_d.ap()[:, s0:s0 + XSL])
            for c in range(QKC):
                nc.sync.dma_start(bq_t[c][:, :], bq_d.ap()[c])
                nc.sync.dma_start(bk_t[c][:, :], bk_d.ap()[c])
            nc.sync.dma_start(mb_t[:, :], mb_d.ap()[:, :])
            nc.sync.dma_start(tri_t[:, :], tri_d.ap()[:, :])
            nc.gpsimd.dma_start(wv_all[:, :], wv_d.ap()[:, :])
            nc.gpsimd.dma_start(bvr_t[:, :], bvr_d.ap()[:, :])

            def proj_qk_chain(c, t0, which):
                # one 512-col chain of the qT or kT projection for chunk c
                w_all, dst, bias = ((wk_all, kt, bk_t), (wq_all, qt, bq_t))[which]
                ps = b1_ps.tile([128, 512], F32, tag="b1", name="pp")
                for e in range(ECH):
                    nc.tensor.matmul(
                        ps[:, :],
                        wqk_sl(w_all, c, e),
                        xt_all[:, _xoff(e, t0):_xoff(e, t0) + 512],
                        start=(e == 0), stop=(e == ECH - 1),
                    )
                nc.vector.tensor_scalar_add(
                    dst[c][:, t0:t0 + 512], ps[:, :], bias[c][:, 0:1])

            def proj_v_chain(tb):
                ps = b1_ps.tile([128, OC], F32, tag="b1", name="ppv")
                for e in range(ECH):
                    o = _xoff(e, 128 * tb)
                    nc.tensor.matmul(
                        ps[:, :],
                        xt_all[:, o:o + 128],
                        wv[e][:, :],
                        start=(e == 0), stop=(e == ECH - 1),
                    )
                nc.vector.memset(vt[tb][:, :, D:D + 1], 1.0)
                nc.vector.tensor_tensor(
                    vt[tb][:, :, 0:D],
                    ps.rearrange("p (h d) -> p h d", h=HPC),
                    bvr_t.rearrange("p (h d) -> p h d", h=HPC),
                    op=mybir.AluOpType.add,
                )

            def qk_exp_piece(h0, sb, p, rp, pw):
                # piece p of strip sb for the pair (h0, h0+1): two adjacent
                # matmuls into one psum tile (disjoint row groups -> run
                # concurrently), then exp. Full pieces need ONE 1024-wide exp
                # (same per-key bias for both heads); tail pieces use two.
                c = h0 // 2
                t0 = 128 * sb
                ps = qk_ps.tile([128, 1024], F32, tag="qk", name="qk")
                # 4 matmuls in 64x64 tiling mode: (head half x s half) ->
                # quadrants (col half x partition half) of one psum tile.
                # All four run concurrently in the array (measured 4.3x).
                for ofs, rows in ((0, slice(0, 64)), (512, slice(64, 128))):
                    for so, pr in ((0, slice(0, 64)), (64, slice(64, 128))):
                        nc.tensor.matmul(
                            ps[pr, ofs:ofs + pw],
                            kt[c][rows, t0 + so:t0 + so + 64],
                            qt[c][rows, t0 + rp:t0 + rp + pw],
                            start=True, stop=True,
                        )
                bpos = PAIR_OFF[sb] + 1024 * p
                if pw == 512:
                    nc.scalar.activation(
                        exb[:, bpos:bpos + 1024], ps[:, 0:1024],
                        mybir.ActivationFunctionType.Exp,
                        bias=mb_t[:, sb:sb + 1], scale=SCALE)
                else:
                    nc.scalar.activation(
                        exb[:, bpos:bpos + pw], ps[:, 0:pw],
                        mybir.ActivationFunctionType.Exp,
                        bias=mb_t[:, sb:sb + 1], scale=SCALE)
                    nc.scalar.activation(
                        exb[:, bpos + pw:bpos + 2 * pw], ps[:, 512:512 + pw],
                        mybir.ActivationFunctionType.Exp,
                        bias=mb_t[:, sb:sb + 1], scale=SCALE)

            def tri_strip(h, sb):
                # causal mask on the diagonal 128x128 block (piece 0 holds
                # t_rel [0, pw0) for both heads)
                pw0 = _strip_pieces(sb)[0][1]
                o = PAIR_OFF[sb] + (h % 2) * pw0
                nc.vector.tensor_tensor(
                    exb[:, o:o + 128], exb[:, o:o + 128], tri_t[:, :],
                    op=mybir.AluOpType.mult)

            def pv_chunk(h, tc0, W=TCH):
                ps = b1_ps.tile([D + 1, W], F32, tag="b1", name="pv",
                                padded_shape=[D + 1, TCH])
                last_sb = min((tc0 + W - 1) // 128, NSB - 1)
                segs = []
                for sb in range(last_sb + 1):
                    cs = max(0, 128 * sb - tc0)
                    lo, hi = tc0 + cs - 128 * sb, tc0 + W - 128 * sb
                    for rp, pw in _strip_pieces(sb):
                        s_lo, s_hi = max(lo, rp), min(hi, rp + pw)
                        if s_lo >= s_hi:
                            continue
                        bpos = PAIR_OFF[sb] + 1024 * (rp // 512) + (h % 2) * pw
                        segs.append((sb, bpos + s_lo - rp,
                                     s_lo + 128 * sb - tc0, s_hi - s_lo))
                for i, (sb, rofs, oc0, w) in enumerate(segs):
                    nc.tensor.matmul(
                        ps[:, oc0:oc0 + w],
                        vt[sb][:, h, :],
                        exb[:, rofs:rofs + w],
                        start=(i == 0), stop=(i == len(segs) - 1),
                    )
                st = stage.tile([D + 1, W], F32, tag="st", name="st",
                                padded_shape=[D + 1, TCH])
                nc.vector.tensor_copy(st[:, :], ps[:, :])
                nc.sync.dma_start(out_d.ap()[h, :, tc0:tc0 + W], st[:, :])

            # pv windows: (emit-at-strip, tc0, width). The tail windows are
            # split so most of the last chunk's accumulation runs before the
            # final strips, shrinking the pair-boundary stall.
            PV_WINDOWS = {3: [(0, 512)], 7: [(512, 512)], 11: [(1024, 512)],
                          13: [(1536, 256)], 14: [(1792, 128)],
                          15: [(1920, 128)]}

            def attn_pair(h0, per_strip):
                # Strips ascending; pv chunks inline once prerequisites exist
                # (so the shared expT buffer is fully consumed before the
                # next pair's exps, emitted later, overwrite it).
                h1 = h0 + 1
                for sb in range(NSB):
                    for p, (rp, pw) in enumerate(_strip_pieces(sb)):
                        qk_exp_piece(h0, sb, p, rp, pw)
                    tri_strip(h0, sb)
                    tri_strip(h1, sb)
                    for f in per_strip[sb]:
                        f()
                    for tc0, W in PV_WINDOWS.get(sb, ()):
                        pv_chunk(h0, tc0, W)
                        pv_chunk(h1, tc0, W)

            # ---- pipelined emission ----
            # pair (0,1) with a cascaded start: emit chunk-0 projection chains
            # t-ascending, and after each 512-col chain emit every scores
            # piece whose q/k columns are now available, so the first exp
            # fires as early as possible.
            all_pieces = [(128 * sb + rp + pw, sb, p, rp, pw)
                          for sb in range(NSB)
                          for p, (rp, pw) in enumerate(_strip_pieces(sb))]
            all_pieces.sort(key=lambda x: (x[0], x[1]))
            emitted = set()
            tri_done = set()

            def emit_ready(limit):
                for need, sb, p, rp, pw in all_pieces:
                    if need > limit:
                        break
                    if (sb, p) in emitted:
                        continue
                    qk_exp_piece(0, sb, p, rp, pw)
                    emitted.add((sb, p))
                    if p == 0:
                        tri_strip(0, sb)
                        tri_strip(1, sb)
                        tri_done.add(sb)

            for t0 in range(0, T, 512):
                proj_qk_chain(0, t0, 0)
                proj_qk_chain(0, t0, 1)
                emit_ready(min(t0 + 512, 1024))
            # post-cascade sweep: remaining pieces strip-major + fillers + pv
            f01 = [[] for _ in range(NSB)]
            for tb in range(NSB):
                f01[tb].append(lambda tb=tb: proj_v_chain(tb))
                if tb % 2 == 0:
                    t0, wch = (tb // 2) % 4 * 512, (tb // 2) // 4
                    f01[tb].append(lambda t0=t0, w=wch: proj_qk_chain(1, t0, w))
            for sb in range(NSB):
                for p, (rp, pw) in enumerate(_strip_pieces(sb)):
                    if (sb, p) not in emitted:
                        qk_exp_piece(0, sb, p, rp, pw)
                        emitted.add((sb, p))
                        if p == 0 and sb not in tri_done:
                            tri_strip(0, sb)
                            tri_strip(1, sb)
                            tri_done.add(sb)
                for f in f01[sb]:
                    f()
                for tc0, W in PV_WINDOWS.get(sb, ()):
                    pv_chunk(0, tc0, W)
                    pv_chunk(1, tc0, W)
            # pair (2,3) fillers: chunk-2 q/k projections
            f23 = [[] for _ in range(NSB)]
            for i in range(8):
                t0, wch = (i % 4) * 512, i // 4
                f23[2 * i].append(lambda t0=t0, w=wch: proj_qk_chain(2, t0, w))
            attn_pair(2, f23)
            attn_pair(4, [[] for _ in range(NSB)])

    nc.compile()
    return nc


_NC_CACHE = None


def _get_nc():
    global _NC_CACHE
    if _NC_CACHE is None:
        _NC_CACHE = _build()
    return _NC_CACHE


def _pack_x(xb):
    """[T, E] batch slice -> slice-major packed [128, ECH*T] bf16 (xT layout)."""
    xT = xb.T.reshape(ECH, 128, T // 512, 512)          # [e, p, s, t']
    return np.ascontiguousarray(
        xT.transpose(1, 2, 0, 3).reshape(128, ECH * T)).astype(ml_dtypes.bfloat16)


def _pack_w(w_sl):
    """[384, 768] weight slice -> e-major packed [128, ECH*OC] bf16 (for wv:
    rhs slice for e-chunk at cols [OC*e, OC*(e+1)))."""
    wT = w_sl.T.reshape(ECH, 128, OC)                   # [e, p, j]
    return np.ascontiguousarray(
        wT.transpose(1, 0, 2).reshape(128, ECH * OC)).astype(ml_dtypes.bfloat16)


def _pack_w_cm(w_sl):
    """[384, 768] weight slice -> chunk-major packed [128, ECH*OC] bf16:
    lhsT for (chunk c, e-chunk e) at cols [768c+128e, 768c+128e+128)."""
    wT = w_sl.T.reshape(ECH, 128, QKC, 128)             # [e, p, c, j]
    return np.ascontiguousarray(
        wT.transpose(1, 2, 0, 3).reshape(128, ECH * OC)).astype(ml_dtypes.bfloat16)


def kernel(hidden_states, attention_mask, Wq, bq, Wk, bk, Wv, bv):
    nc = _get_nc()
    in_maps = _make_in_maps(hidden_states, attention_mask, Wq, bq, Wk, bk, Wv, bv)
    res = bass_utils.run_bass_kernel_spmd(nc, in_maps, core_ids=list(range(NCORES)))
    return _assemble(res.results)


def _make_in_maps(hidden_states, attention_mask, Wq, bq, Wk, bk, Wv, bv):
    hidden_states = np.asarray(hidden_states, dtype=np.float32)
    attention_mask = np.asarray(attention_mask, dtype=np.float32)
    Wq, Wk, Wv = (np.asarray(w, dtype=np.float32) for w in (Wq, Wk, Wv))
    bq, bk, bv = (np.asarray(b, dtype=np.float32) for b in (bq, bk, bv))

    tri = np.tril(np.ones((128, 128), np.float32)).T.astype(ml_dtypes.bfloat16)
    # tri[s, t] = 1 where t >= s

    in_maps = []
    for c in range(NCORES):
        b, g = divmod(c, 2)
        sl = slice(OC * g, OC * (g + 1))
        m = attention_mask[b, 0, 0, :]
        in_maps.append({
            "xTp": _pack_x(hidden_states[b]),
            "wqp": _pack_w_cm(Wq[sl]),
            "wkp": _pack_w_cm(Wk[sl]),
            "wvp": _pack_w(Wv[sl]),
            "bq": np.ascontiguousarray(bq[sl]).reshape(QKC, 128, 1),
            "bk": np.ascontiguousarray(bk[sl]).reshape(QKC, 128, 1),
            "bvr": np.broadcast_to(bv[sl], (128, OC)).copy(),
            "mb": np.ascontiguousarray(m.reshape(NSB, 128).T),
            "tri": tri,
        })
    return in_maps


def _assemble(results):
    out = np.empty((B, T, E), np.float32)
    for c in range(NCORES):
        b, g = divmod(c, 2)
        oT = results[c]["outT"]  # [6, 65, 2048]
        for h6 in range(HPC):
            h = HPC * g + h6
            out[b, :, D * h:D * h + D] = (oT[h6, :D] / oT[h6, D:D + 1]).T
    return out


# revision 39
# speedup vs baseline: 1.0447x; 1.0447x over previous
"""Causal self-attention (B=4, T=2048, E=768, H=12, D=64) on 8 TRN2 NeuronCores.

Sharding: core c handles batch b = c//2 and head-group g = c%2 (6 heads each).
Per core:
    qT, kT = (x @ WqT + bq).T, ...        stored [384, 2048] (heads x 64, T)
    v      = x @ WvT + bv                 stored [2048, 6, 65] with a ones col
    per head-pair, per key-block sb (128 keys), per 512-col t piece:
        scoresT[s, t] = kT_h[:, s]·qT_h[:, t] for BOTH heads into ONE psum
        tile [128, 1024] (h0 at cols 0-511, h1 at 512-1023). The two matmuls
        are adjacent and touch disjoint PE row groups (rows 0-63 / 64-127),
        so the systolic array runs them concurrently (measured 2x).
        ONE exp op covers both heads' halves (same per-key mask bias), writing
        a pair-interleaved bf16 expT buffer.
    per head, per 512-col t-chunk:
        outT[d_aug, t] += v_aug[s, :].T @ expT[s, t]  (PE accumulate, M=65)
    outT row 64 = softmax denominator (ones column of v_aug).
Host: output[b, :, h*64:(h+1)*64] = (outT_h[:64] / outT_h[64:65]).T

Tail pieces of each key strip are extended backward to a full 512 columns
(recomputing a small overlap) so every psum piece is full -> exp ops stay
1024 wide. Strips shorter than 512 use two exp ops (gap in psum).
All matmul writes start on a PSUM bank boundary (mid-bank start=True writes
hang the hardware). Inputs are host-packed into the exact SBUF layouts so
every input lands in a handful of contiguous 2D DMAs on two queues.
"""

import numpy as np
import ml_dtypes

import concourse.bacc as bacc
import concourse.mybir as mybir
import concourse.tile as tile
from concourse import bass_utils

F32 = mybir.dt.float32
BF16 = mybir.dt.bfloat16

B, T, E, H, D = 4, 2048, 768, 12, 64
NCORES = 8
HPC = 6             # heads per core
OC = HPC * D        # 384 output channels per core
ECH = E // 128      # 6 contraction chunks
QKC = OC // 128     # 3 qT/kT partition chunks (= head pairs)
NSB = T // 128      # 16 key blocks
SCALE = 0.125       # 1/sqrt(D)
TCH = 512           # PV t-chunk width
XSL = ECH * 512     # xT free elems per t-slice (3072)


def _strip_pieces(sb):
    """Non-overlapping (start, width) pieces of strip sb on the 512 grid."""
    W = T - 128 * sb
    return [(p, min(512, W - p)) for p in range(0, W, 512)]


# pair-interleaved expT buffer layout: strip sb at PAIR_OFF[sb]; piece p at
# PAIR_OFF[sb] + 1024*p, holding h0's pw cols then h1's pw cols
PAIR_OFF = [0] * (NSB + 1)
for _sb in range(NSB):
    PAIR_OFF[_sb + 1] = PAIR_OFF[_sb] + 2 * (T - 128 * _sb)
PAIRW = PAIR_OFF[NSB]  # 34816


def _xoff(e, t):
    """Free-dim offset of (e-chunk, t) in the slice-major packed xT tile.
    Valid for ranges within one 512-wide t slice."""
    return (t // 512) * XSL + 512 * e + (t % 512)


def _build():
    nc = bacc.Bacc("TRN2", debug=False)

    xT_d = nc.dram_tensor("xTp", [128, ECH * T], BF16, kind="ExternalInput")
    wq_d = nc.dram_tensor("wqp", [128, ECH * OC], BF16, kind="ExternalInput")
    wk_d = nc.dram_tensor("wkp", [128, ECH * OC], BF16, kind="ExternalInput")
    wv_d = nc.dram_tensor("wvp", [128, ECH * OC], BF16, kind="ExternalInput")
    bq_d = nc.dram_tensor("bq", [QKC, 128, 1], F32, kind="ExternalInput")
    bk_d = nc.dram_tensor("bk", [QKC, 128, 1], F32, kind="ExternalInput")
    bvr_d = nc.dram_tensor("bvr", [128, OC], F32, kind="ExternalInput")
    mb_d = nc.dram_tensor("mb", [128, NSB], F32, kind="ExternalInput")
    tri_d = nc.dram_tensor("tri", [128, 128], BF16, kind="ExternalInput")
    out_d = nc.dram_tensor("outT", [HPC, D + 1, T], F32, kind="ExternalOutput")

    with tile.TileContext(nc) as tc:
        with (
            tc.tile_pool(name="persist", bufs=1) as pp,
            tc.tile_pool(name="qk_ps", bufs=3, space="PSUM") as qk_ps,
            tc.tile_pool(name="b1_ps", bufs=2, space="PSUM") as b1_ps,
            tc.tile_pool(name="stage", bufs=4) as stage,
        ):
            # ---- persistent SBUF tensors ----
            xt_all = pp.tile([128, ECH * T], BF16, tag="xt", name="xt")
            wq_all = pp.tile([128, ECH * OC], BF16, tag="wq", name="wq")
            wk_all = pp.tile([128, ECH * OC], BF16, tag="wk", name="wk")
            wv_all = pp.tile([128, ECH * OC], BF16, tag="wv", name="wv")
            # wq/wk are packed chunk-major: lhsT for (c, e) at cols 768c+128e
            wv = [wv_all[:, OC * e:OC * (e + 1)] for e in range(ECH)]

            def wqk_sl(w_all, c, e):
                o = 768 * c + 128 * e
                return w_all[:, o:o + 128]
            qt = [pp.tile([128, T], BF16, tag=f"qt{c}", name=f"qt{c}") for c in range(QKC)]
            kt = [pp.tile([128, T], BF16, tag=f"kt{c}", name=f"kt{c}") for c in range(QKC)]
            vt = [pp.tile([128, HPC, D + 1], BF16, tag=f"vt{t}", name=f"vt{t}") for t in range(NSB)]
            exb = pp.tile([128, PAIRW], BF16, tag="exb", name="exb")
            bq_t = [pp.tile([128, 1], F32, tag=f"bq{c}", name=f"bq{c}") for c in range(QKC)]
            bk_t = [pp.tile([128, 1], F32, tag=f"bk{c}", name=f"bk{c}") for c in range(QKC)]
            bvr_t = pp.tile([128, OC], F32, tag="bvr", name="bvr")
            mb_t = pp.tile([128, NSB], F32, tag="mb", name="mb")
            tri_t = pp.tile([128, 128], BF16, tag="tri", name="tri")

            # ---- input DMAs: contiguous 2D transfers on two hardware queues
            # (sync + scalar); xT slice-major so projections start early ----
            for c in range(QKC):
                nc.sync.dma_start(wk_all[:, 768 * c:768 * c + 768],
                                  wk_d.ap()[:, 768 * c:768 * c + 768])
                nc.sync.dma_start(wq_all[:, 768 * c:768 * c + 768],
                                  wq_d.ap()[:, 768 * c:768 * c + 768])
            for t0 in (0, 512, 1024, 1536):
                s0 = (t0 // 512) * XSL
                nc.scalar.dma_start(
                    xt_all[:, s0:s0 + XSL], xT_d.ap()[:, s0:s0 + XSL])
            for c in range(QKC):
                nc.sync.dma_start(bq_t[c][:, :], bq_d.ap()[c])
                nc.sync.dma_start(bk_t[c][:, :], bk_d.ap()[c])
            nc.sync.dma_start(mb_t[:, :], mb_d.ap()[:, :])
            nc.sync.dma_start(tri_t[:, :], tri_d.ap()[:, :])
            nc.gpsimd.dma_start(wv_all[:, :], wv_d.ap()[:, :])
            nc.gpsimd.dma_start(bvr_t[:, :], bvr_d.ap()[:, :])

            def proj_qk_chain(c, t0, which):
                # one 512-col chain of the qT or kT projection for chunk c
                w_all, dst, bias = ((wk_all, kt, bk_t), (wq_all, qt, bq_t))[which]
                ps = b1_ps.tile([128, 512], F32, tag="b1", name="pp")
                for e in range(ECH):
                    nc.tensor.matmul(
                        ps[:, :],
                        wqk_sl(w_all, c, e),
                        xt_all[:, _xoff(e, t0):_xoff(e, t0) + 512],
                        start=(e == 0), stop=(e == ECH - 1),
                    )
                nc.vector.tensor_scalar_add(
                    dst[c][:, t0:t0 + 512], ps[:, :], bias[c][:, 0:1])

            def proj_v_chain(tb):
                ps = b1_ps.tile([128, OC], F32, tag="b1", name="ppv")
                for e in range(ECH):
                    o = _xoff(e, 128 * tb)
                    nc.tensor.matmul(
                        ps[:, :],
                        xt_all[:, o:o + 128],
                        wv[e][:, :],
                        start=(e == 0), stop=(e == ECH - 1),
                    )
                nc.vector.memset(vt[tb][:, :, D:D + 1], 1.0)
                nc.vector.tensor_tensor(
                    vt[tb][:, :, 0:D],
                    ps.rearrange("p (h d) -> p h d", h=HPC),
                    bvr_t.rearrange("p (h d) -> p h d", h=HPC),
                    op=mybir.AluOpType.add,
                )

            def qk_exp_piece(h0, sb, p, rp, pw):
                # piece p of strip sb for the pair (h0, h0+1): two adjacent
                # matmuls into one psum tile (disjoint row groups -> run
                # concurrently), then exp. Full pieces need ONE 1024-wide exp
                # (same per-key bias for both heads); tail pieces use two.
                c = h0 // 2
                t0 = 128 * sb
                ps = qk_ps.tile([128, 1024], F32, tag="qk", name="qk")
                # 4 matmuls in 64x64 tiling mode: (head half x s half) ->
                # quadrants (col half x partition half) of one psum tile.
                # All four run concurrently in the array (measured 4.3x).
                for ofs, rows in ((0, slice(0, 64)), (512, slice(64, 128))):
                    for so, pr in ((0, slice(0, 64)), (64, slice(64, 128))):
                        nc.tensor.matmul(
                            ps[pr, ofs:ofs + pw],
                            kt[c][rows, t0 + so:t0 + so + 64],
                            qt[c][rows, t0 + rp:t0 + rp + pw],
                            start=True, stop=True,
                        )
                bpos = PAIR_OFF[sb] + 1024 * p
                if pw == 512:
                    nc.scalar.activation(
                        exb[:, bpos:bpos + 1024], ps[:, 0:1024],
                        mybir.ActivationFunctionType.Exp,
                        bias=mb_t[:, sb:sb + 1], scale=SCALE)
                else:
                    nc.scalar.activation(
                        exb[:, bpos:bpos + pw], ps[:, 0:pw],
                        mybir.ActivationFunctionType.Exp,
                        bias=mb_t[:, sb:sb + 1], scale=SCALE)
                    nc.scalar.activation(
                        exb[:, bpos + pw:bpos + 2 * pw], ps[:, 512:512 + pw],
                        mybir.ActivationFunctionType.Exp,
                        bias=mb_t[:, sb:sb + 1], scale=SCALE)

            def tri_strip(h, sb):
                # causal mask on the diagonal 128x128 block (piece 0 holds
                # t_rel [0, pw0) for both heads)
                pw0 = _strip_pieces(sb)[0][1]
                o = PAIR_OFF[sb] + (h % 2) * pw0
                nc.vector.tensor_tensor(
                    exb[:, o:o + 128], exb[:, o:o + 128], tri_t[:, :],
                    op=mybir.AluOpType.mult)

            def pv_chunk(h, tc0, W=TCH):
                ps = b1_ps.tile([D + 1, W], F32, tag="b1", name="pv",
                                padded_shape=[D + 1, TCH])
                last_sb = min((tc0 + W - 1) // 128, NSB - 1)
                segs = []
                for sb in range(last_sb + 1):
                    cs = max(0, 128 * sb - tc0)
                    lo, hi = tc0 + cs - 128 * sb, tc0 + W - 128 * sb
                    for rp, pw in _strip_pieces(sb):
                        s_lo, s_hi = max(lo, rp), min(hi, rp + pw)
                        if s_lo >= s_hi:
                            continue
                        bpos = PAIR_OFF[sb] + 1024 * (rp // 512) + (h % 2) * pw
                        segs.append((sb, bpos + s_lo - rp,
                                     s_lo + 128 * sb - tc0, s_hi - s_lo))
                for i, (sb, rofs, oc0, w) in enumerate(segs):
                    nc.tensor.matmul(
                        ps[:, oc0:oc0 + w],
                        vt[sb][:, h, :],
                        exb[:, rofs:rofs + w],
                        start=(i == 0), stop=(i == len(segs) - 1),
                    )
                st = stage.tile([D + 1, W], F32, tag="st", name="st",
                                padded_shape=[D + 1, TCH])
                nc.vector.tensor_copy(st[:, :], ps[:, :])
                nc.sync.dma_start(out_d.ap()[h, :, tc0:tc0 + W], st[:, :])

            # pv windows: (emit-at-strip, tc0, width). The tail windows are
            # split so most of the last chunk's accumulation runs before the
            # final strips, shrinking the pair-boundary stall.
            PV_WINDOWS = {3: [(0, 512)], 7: [(512, 512)], 11: [(1024, 512)],
                          13: [(1536, 256)], 14: [(1792, 128)],
                          15: [(1920, 128)]}

            def attn_pair(h0, per_strip):
                # Strips ascending; pv chunks inline once prerequisites exist
                # (so the shared expT buffer is fully consumed before the
                # next pair's exps, emitted later, overwrite it).
                h1 = h0 + 1
                for sb in range(NSB):
                    for p, (rp, pw) in enumerate(_strip_pieces(sb)):
                        qk_exp_piece(h0, sb, p, rp, pw)
                    tri_strip(h0, sb)
                    tri_strip(h1, sb)
                    for f in per_strip[sb]:
                        f()
                    for tc0, W in PV_WINDOWS.get(sb, ()):
                        pv_chunk(h0, tc0, W)
                        pv_chunk(h1, tc0, W)

            # ---- pipelined emission: a global slot pipeline ----
            # Pair p's strips occupy slots [12p, 12p+16); consecutive pairs
            # overlap by 4 slots so ACT never idles at pair boundaries. A
            # piece whose expT region is still to be read by the previous
            # pair's late pv windows (t >= 1536) is deferred past them.
            NSLOT = 41
            slot_work = [[] for _ in range(NSLOT)]

            # pair (0,1) cascaded start: emit chunk-0 projection chains
            # t-ascending; after each chain emit every scores piece whose q/k
            # columns are available, so the first exp fires early.
            all_pieces = [(128 * sb + rp + pw, sb, p, rp, pw)
                          for sb in range(NSB)
                          for p, (rp, pw) in enumerate(_strip_pieces(sb))]
            all_pieces.sort(key=lambda x: (x[0], x[1]))
            emitted = set()
            tri_done = set()

            def emit_ready(limit):
                for need, sb, p, rp, pw in all_pieces:
                    if need > limit:
                        break
                    if (sb, p) in emitted:
                        continue
                    qk_exp_piece(0, sb, p, rp, pw)
                    emitted.add((sb, p))
                    if p == 0:
                        tri_strip(0, sb)
                        tri_strip(1, sb)
                        tri_done.add(sb)

            for t0 in range(0, T, 512):
                proj_qk_chain(0, t0, 0)
                proj_qk_chain(0, t0, 1)
                emit_ready(min(t0 + 512, 1024))

            def sched_pair(h0, base, defer_slot):
                for sb in range(NSB):
                    for p, (rp, pw) in enumerate(_strip_pieces(sb)):
                        if h0 == 0 and (sb, p) in emitted:
                            continue
                        t_end = 128 * sb + rp + pw
                        sl = base + sb
                        if t_end > 1536 and sl < defer_slot:
                            sl = defer_slot
                        work = [lambda h0=h0, sb=sb, p=p, rp=rp, pw=pw:
                                qk_exp_piece(h0, sb, p, rp, pw)]
                        if p == 0 and not (h0 == 0 and sb in tri_done):
                            work.append(lambda h0=h0, sb=sb: tri_strip(h0, sb))
                            work.append(lambda h0=h0, sb=sb: tri_strip(h0 + 1, sb))
                        slot_work[sl].extend(work)
                    for tc0, W in PV_WINDOWS.get(sb, ()):
                        sl = max(base + sb, defer_slot if base else 0)
                        slot_work[sl].append(
                            lambda h0=h0, tc0=tc0, W=W: pv_chunk(h0, tc0, W))
                        slot_work[sl].append(
                            lambda h0=h0, tc0=tc0, W=W: pv_chunk(h0 + 1, tc0, W))

            sched_pair(0, 0, 0)
            sched_pair(2, 12, 16)
            sched_pair(4, 24, 28)
            # fillers: v projection paced over pair01's strips (1/slot, just
            # ahead of the pv windows); chunk-1/2 q/k projections early
            for tb in range(NSB):
                slot_work[tb].insert(0, lambda tb=tb: proj_v_chain(tb))
            for i in range(8):
                t0, wch = (i % 4) * 512, i // 4
                slot_work[1 + i].append(
                    lambda t0=t0, w=wch: proj_qk_chain(1, t0, w))
                slot_work[13 + i].append(
                    lambda t0=t0, w=wch: proj_qk_chain(2, t0, w))

            for sl in range(NSLOT):
                for f in slot_work[sl]:
                    f()

    nc.compile()
    return nc


_NC_CACHE = None


def _get_nc():
    global _NC_CACHE
    if _NC_CACHE is None:
        _NC_CACHE = _build()
    return _NC_CACHE


def _pack_x(xb):
    """[T, E] batch slice -> slice-major packed [128, ECH*T] bf16 (xT layout)."""
    xT = xb.T.reshape(ECH, 128, T // 512, 512)          # [e, p, s, t']
    return np.ascontiguousarray(
        xT.transpose(1, 2, 0, 3).reshape(128, ECH * T)).astype(ml_dtypes.bfloat16)


def _pack_w(w_sl):
    """[384, 768] weight slice -> e-major packed [128, ECH*OC] bf16 (for wv:
    rhs slice for e-chunk at cols [OC*e, OC*(e+1)))."""
    wT = w_sl.T.reshape(ECH, 128, OC)                   # [e, p, j]
    return np.ascontiguousarray(
        wT.transpose(1, 0, 2).reshape(128, ECH * OC)).astype(ml_dtypes.bfloat16)


def _pack_w_cm(w_sl):
    """[384, 768] weight slice -> chunk-major packed [128, ECH*OC] bf16:
    lhsT for (chunk c, e-chunk e) at cols [768c+128e, 768c+128e+128)."""
    wT = w_sl.T.reshape(ECH, 128, QKC, 128)             # [e, p, c, j]
    return np.ascontiguousarray(
        wT.transpose(1, 2, 0, 3).reshape(128, ECH * OC)).astype(ml_dtypes.bfloat16)


def kernel(hidden_states, attention_mask, Wq, bq, Wk, bk, Wv, bv):
    nc = _get_nc()
    in_maps = _make_in_maps(hidden_states, attention_mask, Wq, bq, Wk, bk, Wv, bv)
    res = bass_utils.run_bass_kernel_spmd(nc, in_maps, core_ids=list(range(NCORES)))
    return _assemble(res.results)


def _make_in_maps(hidden_states, attention_mask, Wq, bq, Wk, bk, Wv, bv):
    hidden_states = np.asarray(hidden_states, dtype=np.float32)
    attention_mask = np.asarray(attention_mask, dtype=np.float32)
    Wq, Wk, Wv = (np.asarray(w, dtype=np.float32) for w in (Wq, Wk, Wv))
    bq, bk, bv = (np.asarray(b, dtype=np.float32) for b in (bq, bk, bv))

    tri = np.tril(np.ones((128, 128), np.float32)).T.astype(ml_dtypes.bfloat16)
    # tri[s, t] = 1 where t >= s

    in_maps = []
    for c in range(NCORES):
        b, g = divmod(c, 2)
        sl = slice(OC * g, OC * (g + 1))
        m = attention_mask[b, 0, 0, :]
        in_maps.append({
            "xTp": _pack_x(hidden_states[b]),
            "wqp": _pack_w_cm(Wq[sl]),
            "wkp": _pack_w_cm(Wk[sl]),
            "wvp": _pack_w(Wv[sl]),
            "bq": np.ascontiguousarray(bq[sl]).reshape(QKC, 128, 1),
            "bk": np.ascontiguousarray(bk[sl]).reshape(QKC, 128, 1),
            "bvr": np.broadcast_to(bv[sl], (128, OC)).copy(),
            "mb": np.ascontiguousarray(m.reshape(NSB, 128).T),
            "tri": tri,
        })
    return in_maps


def _assemble(results):
    out = np.empty((B, T, E), np.float32)
    for c in range(NCORES):
        b, g = divmod(c, 2)
        oT = results[c]["outT"]  # [6, 65, 2048]
        for h6 in range(HPC):
            h = HPC * g + h6
            out[b, :, D * h:D * h + D] = (oT[h6, :D] / oT[h6, D:D + 1]).T
    return out
